# revision 1
# baseline (speedup 1.0000x reference)
"""Distributed trace-polynomial kernel for trn2 (8 NeuronCores).

Problem: x [65536,16,16], coef [10,4].
  t_i(b) = trace(x_b^(i+2)), i=0..9
  out[b] = sum_ij coef[i,j] * t_i^(j+1) / 256^(i+j+1)

Optimization vs reference (10 chained matmuls): only powers
x^2,x^3,x^4,x^5,x^6 are materialized (5 batched matmuls via the
addition chain 2=1+1, 3=2+1, 4=2+2, 5=4+1, 6=4+2); traces of
x^7..x^11 come from Frobenius pairs tr(x^(a+b)) = <x^a, (x^b)^T>.

Sharding: pure data parallel — B=65536 split 8 ways across the
8 NeuronCores (jax pmap / PJRT on the axon-tunneled trn2 chip),
coef replicated. Output gathered to the full [65536] vector.
"""

import numpy as np
import jax
import jax.numpy as jnp

B, N = 65536, 16
ROWS, COLS = 10, 4
M = 8  # cores
BS = B // M  # 8192 per core

_compiled = None


def _shard_fn(x, coef):
    # x: [BS, 16, 16]  coef: [10, 4]
    x2 = jnp.matmul(x, x)
    x3 = jnp.matmul(x2, x)
    x4 = jnp.matmul(x2, x2)
    x5 = jnp.matmul(x4, x)
    x6 = jnp.matmul(x4, x2)

    def tr(m):
        return jnp.trace(m, axis1=-2, axis2=-1)

    def frob(a, b):  # trace(a @ b) without the matmul
        return jnp.sum(a * jnp.swapaxes(b, -1, -2), axis=(-1, -2))

    t = jnp.stack(
        [
            tr(x2),          # k=2
            tr(x3),          # k=3
            tr(x4),          # k=4
            tr(x5),          # k=5
            tr(x6),          # k=6
            frob(x3, x4),    # k=7
            frob(x4, x4),    # k=8
            frob(x4, x5),    # k=9
            frob(x5, x5),    # k=10
            frob(x5, x6),    # k=11
        ],
        axis=-1,
    )  # [BS, 10]

    n = jnp.float32(N * N)  # 256
    s = t / n  # [BS, 10]
    # out[b] = sum_i n^-i * sum_j coef[i,j] * s_i^(j+1)
    jexp = jnp.arange(1, COLS + 1, dtype=jnp.float32)         # [4]
    iexp = jnp.arange(ROWS, dtype=jnp.float32)                 # [10]
    feats = s[:, :, None] ** jexp[None, None, :]               # [BS, 10, 4]
    w = coef * (n ** (-iexp))[:, None]                         # [10, 4]
    return jnp.einsum("rc,brc->b", w, feats)                   # [BS]


def _get_compiled():
    global _compiled
    if _compiled is None:
        devs = jax.devices()[:M]
        _compiled = jax.pmap(_shard_fn, axis_name="p", devices=devs)
    return _compiled


def kernel(x: np.ndarray, coef: np.ndarray) -> np.ndarray:
    x = np.ascontiguousarray(x, dtype=np.float32)
    coef = np.ascontiguousarray(coef, dtype=np.float32)
    xs = x.reshape(M, BS, N, N)                      # shard batch across 8 cores
    cs = np.broadcast_to(coef, (M, ROWS, COLS))      # replicate coef
    fn = _get_compiled()
    out = fn(xs, cs)                                 # [8, 8192]
    return np.asarray(out, dtype=np.float32).reshape(B)
